# revision 18
# baseline (speedup 1.0000x reference)
"""Distributed GINE stack (3 layers) on 8 TRN2 NeuronCores.

Self-contained: takes FULL inputs, shards internally, runs one SPMD Bass
program on cores 0-7 via run_bass_kernel_spmd, gathers the full output.

Strategy (dst-sharded graph partitioning), v2:
  - nodes padded to NP, core c owns `shard` nodes = G groups of 128
  - edges assigned to the core owning dst; per core, edges are grouped by
    (dst-group, src-sub-shard) and padded to 128-edge tiles
  - node table is split into TWO interleaved sub-shards (A = groups 0..24,
    B = groups 25..48 of every core) so each AllGather is smaller and the
    second overlaps the next layer's phase-A gathers; sub-shard tables also
    keep int16 gather indices < 32768
  - gathers run on 4 SWDGE queues round-robin -> 4 Q7 core pairs generate
    descriptors concurrently (the single-queue path was the bottleneck)
  - bias be[l] is folded into the gathered tables (host adds it to sg0 /
    device adds it when writing ag inputs), so the edge matmul contracts
    K=64 exactly and even/odd edge tiles are packed into the top/bottom
    halves of the PE array (tile_position row tiling)
  - e = ea @ We via PE (edge_attr^T resident in SBUF as fp8 in two stacked
    64-row halves, x16 weight prescale to dodge fp8 subnormals)
  - msg = relu(e/16 + s_gath): DVE scalar_tensor_tensor then ScalarE relu
  - scatter-add via PE one-hot matmuls with fp8 one-hot tiles (exact)
  - each layer: edge sweep (agg for all groups, spilled to SBUF fp32), then
    MLP sweep, then batched LayerNorm (bn_stats/bn_aggr on DVE, one Sqrt
    batch on ScalarE) and final SiLU sweep - ScalarE activation-table
    reloads drop from ~100/layer to ~4/layer
"""

import os
import sys

for _p in (
    "/root/.axon_site",
    "/root/.axon_site/_ro/trn_rl_repo",
    "/root/.axon_site/_ro/pypackages",
    "/opt/trn_rl_repo",
    "/opt/pypackages",
):
    if os.path.isdir(_p) and _p not in sys.path:
        sys.path.append(_p)

import numpy as np
import ml_dtypes

BF16 = ml_dtypes.bfloat16
FP8 = ml_dtypes.float8_e4m3

H = 128
ED = 64
L = 3
LN_EPS = 1e-5
N_CORES = 8
GN = 128            # nodes per aggregation group
WE_SCALE = 16.0     # fp8 prescale on We
CHT = 8             # edge tiles per gather chunk (8*128 = 1024 idxs)
GA = 25             # groups per core in sub-shard A (B gets G - GA)
NQ = 4              # SWDGE queues used round-robin for gathers


# ================================================================ CPU planning
class Plan:
    pass


def build_plan(edge_index, n_nodes, n_cores=N_CORES, gn=GN):
    src = np.asarray(edge_index[0]).astype(np.int64)
    dst = np.asarray(edge_index[1]).astype(np.int64)

    G = int(np.ceil(n_nodes / (n_cores * gn)))
    shard = G * gn
    NP = shard * n_cores
    ga = min(GA, (G + 1) // 2)
    gb = G - ga
    rowsA, rowsB = ga * gn, gb * gn
    NA, NB = rowsA * n_cores, rowsB * n_cores
    assert NA <= 32768 and NB <= 32768, (NA, NB)

    owner = dst // shard
    counts = np.zeros((n_cores, 2, G), dtype=np.int64)
    order = []
    for c in range(n_cores):
        m_c = owner == c
        s_c, d_c = src[m_c], dst[m_c]
        g_c = (d_c - c * shard) // gn
        r_c = s_c % shard
        ph_c = (r_c >= rowsA).astype(np.int64)
        key = (g_c * 2 + ph_c) * NP + d_c
        o = np.argsort(key, kind="stable")
        order.append((np.nonzero(m_c)[0][o], s_c[o], d_c[o], ph_c[o], g_c[o]))
        np.add.at(counts[c], (ph_c, g_c), 1)

    T = np.zeros((2, G), dtype=np.int64)
    for p in range(2):
        for g in range(G):
            T[p, g] = int(np.ceil(counts[:, p, g].max() / 128))

    # stream-major schedule: phase-A tiles for all groups first, then phase-B
    sched = []
    t0 = 0
    for p in range(2):
        for g in range(G):
            if T[p, g] > 0:
                sched.append((g, p, t0, int(T[p, g])))
                t0 += int(T[p, g])
    TtotA = int(T[0].sum())
    Ttot = t0
    ECpad = Ttot * 128
    EAW = ((Ttot + 1) // 2) * 128

    plan = Plan()
    plan.EAW = EAW
    plan.n_cores, plan.gn, plan.G = n_cores, gn, G
    plan.ga, plan.gb = ga, gb
    plan.rowsA, plan.rowsB, plan.NA, plan.NB = rowsA, rowsB, NA, NB
    plan.shard, plan.NP = shard, NP
    plan.T, plan.sched, plan.Ttot, plan.ECpad = T, sched, Ttot, ECpad
    plan.TtotA = TtotA

    plan.perm, plan.gidx, plan.dstloc, plan.srcs = [], [], [], []
    for c in range(n_cores):
        idx_c, s_c, d_c, ph_c, g_c = order[c]
        perm = np.full(ECpad, -1, dtype=np.int64)
        gidx = np.zeros(ECpad, dtype=np.int16)
        dstloc = np.full(ECpad, -1, dtype=np.int64)
        srcs = np.zeros(ECpad, dtype=np.int64)
        for (g, p, ts, nt) in sched:
            m = (g_c == g) & (ph_c == p)
            k = int(m.sum())
            assert k <= nt * 128
            sl = slice(ts * 128, ts * 128 + k)
            perm[sl] = idx_c[m]
            s_m = s_c[m]
            r_m = s_m % shard
            c_m = s_m // shard
            if p == 0:
                gidx[sl] = (c_m * rowsA + r_m).astype(np.int16)
            else:
                gidx[sl] = (c_m * rowsB + (r_m - rowsA)).astype(np.int16)
            dstloc[sl] = d_c[m] - c * shard - g * gn
            srcs[sl] = s_m
        plan.perm.append(perm)
        plan.gidx.append(gidx)
        plan.dstloc.append(dstloc)
        plan.srcs.append(srcs)
    return plan


def build_core_inputs(plan, c, s, edge_attr, We, be, W1, b1, W2, b2,
                      gamma, beta, trivial_ln):
    shard, ECpad, Ttot = plan.shard, plan.ECpad, plan.Ttot
    perm, dstloc = plan.perm[c], plan.dstloc[c]
    n = s.shape[0]
    ed = edge_attr.shape[1]
    nl = We.shape[0]

    s0_shard = np.zeros((shard, H), dtype=np.float32)
    lo, hi = c * shard, min((c + 1) * shard, n)
    if hi > lo:
        s0_shard[: hi - lo] = s[lo:hi]

    # edge_attr^T packed as two stacked 64-row halves: tile t lives at
    # partitions 64*(t%2) .. +64, columns (t//2)*128 .. +128
    real = perm >= 0
    eaT = np.zeros((ed, ECpad), dtype=FP8)
    eaT[:, real] = edge_attr[perm[real]].T.astype(FP8)
    ea2 = np.zeros((128, plan.EAW), dtype=FP8)
    for par in range(2):
        # tiles t with t%2 == par -> columns [(t//2)*128, (t//2)*128+128)
        sel = np.arange(ECpad).reshape(Ttot, 128)[par::2].reshape(-1)
        ea2[par * 64:(par + 1) * 64, :sel.size] = eaT[:, sel]

    gidx = plan.gidx[c]
    gw = gidx.reshape(ECpad // 16, 16).T          # [16, ECpad/16]
    gidx_w = np.ascontiguousarray(np.tile(gw, (8, 1)))  # [128, ECpad/16]

    S = np.zeros((128, Ttot * 128), dtype=FP8)
    slot = np.arange(ECpad)
    ok = dstloc >= 0
    S[slot[ok] % 128, (slot[ok] // 128) * 128 + dstloc[ok]] = np.float32(1.0)

    # per (layer, tile-parity) weight blocks: even tiles contract rows 0-63,
    # odd tiles rows 64-127; the other half is zeros so K=128 matmuls work
    webe2 = np.zeros((128, 2 * nl * H), dtype=FP8)
    for l in range(nl):
        w = (We[l] * WE_SCALE).astype(FP8)
        webe2[:ed, (2 * l) * H:(2 * l + 1) * H] = w
        webe2[ed:2 * ed, (2 * l + 1) * H:(2 * l + 2) * H] = w

    w1 = np.concatenate([W1[l] for l in range(nl)], axis=1).astype(BF16)
    w2 = np.concatenate([W2[l] for l in range(nl)], axis=1).astype(BF16)
    b1c = np.stack([b1[l] for l in range(nl)], axis=1).astype(np.float32)
    b2r = np.concatenate([b2[l].reshape(1, H) for l in range(nl)], axis=1).astype(BF16)

    # host pre-gather of layer-0 messages: rows of (s + be[0])
    table0 = np.zeros((plan.NP, H), dtype=np.float32)
    table0[:n] = s + be[0]
    srcs = plan.srcs[c]
    slot_all = np.arange(ECpad)
    sg0 = np.zeros((128, Ttot, H), dtype=BF16)
    sg0[slot_all % 128, slot_all // 128, :] = table0[srcs].astype(BF16)
    sg0 = np.ascontiguousarray(sg0.reshape(128, Ttot * H))

    inp = {
        "sg0": sg0,
        "s0": s0_shard,
        "ea2": ea2,
        "gidx": gidx_w,
        "smat": S,
        "webe2": webe2,
        "w1": np.ascontiguousarray(w1),    # [H, L*H] lhsT blocks
        "w2": np.ascontiguousarray(w2),    # [H, L*H] rhs blocks
        "b1": np.ascontiguousarray(b1c),   # [H, L]
        "b2r": np.ascontiguousarray(b2r),  # [1, L*H]
        "ones1": np.ones((1, 128), dtype=BF16),
        "ident": np.eye(128, dtype=BF16),
    }
    if nl > 1:
        beb = np.concatenate(
            [np.broadcast_to(be[l].reshape(1, H).astype(np.float32), (128, H))
             for l in range(1, nl)], axis=1)
        inp["beb"] = np.ascontiguousarray(beb)
    if not trivial_ln:
        inp["gammab"] = np.ascontiguousarray(np.concatenate(
            [np.broadcast_to(gamma[l].reshape(1, H), (128, H)) for l in range(nl)],
            axis=1).astype(np.float32))
        inp["betab"] = np.ascontiguousarray(np.concatenate(
            [np.broadcast_to(beta[l].reshape(1, H), (128, H)) for l in range(nl)],
            axis=1).astype(np.float32))
    return inp


# ============================================================== device program
def build_program(plan, trivial_ln, ed=ED, n_layers=L, silu_native=True):
    import concourse.bacc as bacc
    import concourse.mybir as mybir
    import concourse.tile as tile
    from concourse.bass import ts as bts

    dt = mybir.dt
    AF = mybir.ActivationFunctionType
    OP = mybir.AluOpType
    AX = mybir.AxisListType

    G, NP, shard = plan.G, plan.NP, plan.shard
    ga, gb = plan.ga, plan.gb
    rowsA, rowsB, NA, NB = plan.rowsA, plan.rowsB, plan.NA, plan.NB
    Ttot, ECpad, sched = plan.Ttot, plan.ECpad, plan.sched
    n_cores = plan.n_cores

    by_group = {}
    for (g, p, ts_, nt) in sched:
        by_group.setdefault(g, {})[p] = (ts_, nt)
    TtotA = plan.TtotA
    stream_rng = {0: (0, TtotA), 1: (TtotA, Ttot)}
    max_nt = max((nt for (_, _, _, nt) in sched), default=1)
    # chunk boundaries aligned to the sub-shard (quadrant) split so no chunk
    # straddles the low/high group boundary (required for buffer recycling)
    tsplit = {}
    for p in (0, 1):
        tsplit[p] = stream_rng[p][1]
        for (g, pp, ts_, nt) in sched:
            if pp == p and g >= ga:
                tsplit[p] = ts_
                break
    chunks = {0: [], 1: []}
    chunk_of = {}  # global tile idx -> (p, chunk idx, offset)
    for p in (0, 1):
        lo, hi = stream_rng[p]
        for (a, b) in ((lo, tsplit[p]), (tsplit[p], hi)):
            t = a
            while t < b:
                nn = min(CHT, b - t)
                k = len(chunks[p])
                for off in range(nn):
                    chunk_of[t + off] = (p, k, off)
                chunks[p].append((t, nn))
                t += nn

    nc = bacc.Bacc("TRN2", target_bir_lowering=False, debug=False,
                   num_devices=n_cores, num_swdge_queues=NQ)

    d_sg0 = nc.dram_tensor("sg0", [128, Ttot * H], dt.bfloat16, kind="ExternalInput")
    d_s0 = nc.dram_tensor("s0", [shard, H], dt.float32, kind="ExternalInput")
    EAW = plan.EAW
    d_ea2 = nc.dram_tensor("ea2", [128, EAW], dt.float8e4, kind="ExternalInput")
    d_gidx = nc.dram_tensor("gidx", [128, ECpad // 16], dt.int16, kind="ExternalInput")
    d_S = nc.dram_tensor("smat", [128, Ttot * 128], dt.float8e4, kind="ExternalInput")
    d_webe2 = nc.dram_tensor("webe2", [128, 2 * n_layers * H], dt.float8e4, kind="ExternalInput")
    d_w1 = nc.dram_tensor("w1", [H, n_layers * H], dt.bfloat16, kind="ExternalInput")
    d_w2 = nc.dram_tensor("w2", [H, n_layers * H], dt.bfloat16, kind="ExternalInput")
    d_b1 = nc.dram_tensor("b1", [H, n_layers], dt.float32, kind="ExternalInput")
    d_b2r = nc.dram_tensor("b2r", [1, n_layers * H], dt.bfloat16, kind="ExternalInput")
    d_ones = nc.dram_tensor("ones1", [1, 128], dt.bfloat16, kind="ExternalInput")
    d_ident = nc.dram_tensor("ident", [128, 128], dt.bfloat16, kind="ExternalInput")
    if n_layers > 1:
        d_beb = nc.dram_tensor("beb", [128, (n_layers - 1) * H], dt.float32,
                               kind="ExternalInput")
    if not trivial_ln:
        d_gb = nc.dram_tensor("gammab", [128, n_layers * H], dt.float32, kind="ExternalInput")
        d_bb = nc.dram_tensor("betab", [128, n_layers * H], dt.float32, kind="ExternalInput")
    d_out = nc.dram_tensor("out", [shard, H], dt.float32, kind="ExternalOutput")

    with tile.TileContext(nc) as tc:
        with (
            tc.tile_pool(name="const", bufs=1) as constp,
            tc.tile_pool(name="dram", bufs=1, space="DRAM") as dramp,
        ):
            def load_const(name, dram, shape, dtype):
                t = constp.tile(shape, dtype, name=name, tag=name)
                nc.sync.dma_start(t[:], dram[:])
                return t

            ea2_sb = load_const("ea2", d_ea2, [128, EAW], dt.float8e4)
            gidx_sb = load_const("gidx", d_gidx, [128, ECpad // 16], dt.int16)
            webe2_sb = load_const("webe2", d_webe2, [128, 2 * n_layers * H], dt.float8e4)
            w1_sb = load_const("w1", d_w1, [H, n_layers * H], dt.bfloat16)
            w2_sb = load_const("w2", d_w2, [H, n_layers * H], dt.bfloat16)
            b1_sb = load_const("b1", d_b1, [H, n_layers], dt.float32)
            b2r_sb = load_const("b2r", d_b2r, [1, n_layers * H], dt.bfloat16)
            ones_sb = load_const("ones1", d_ones, [1, 128], dt.bfloat16)
            ident_sb = load_const("ident", d_ident, [128, 128], dt.bfloat16)
            if n_layers > 1:
                beb_sb = load_const("beb", d_beb, [128, (n_layers - 1) * H], dt.float32)
            if not trivial_ln:
                gb_sb = load_const("gb", d_gb, [128, n_layers * H], dt.float32)
                bb_sb = load_const("bb", d_bb, [128, n_layers * H], dt.float32)

            s_node = constp.tile([128, G * H], dt.float32, name="t_", tag="snode")
            for g in range(G):
                nc.sync.dma_start(s_node[:, bts(g, H)],
                                  d_s0[g * 128:(g + 1) * 128, :])

            agg_sb = constp.tile([128, G * H], dt.float32, name="t_", tag="aggsb")
            veb = constp.tile([128, G], dt.float32, name="t_", tag="veb")
            mv = constp.tile([128, G * 2], dt.float32, name="t_", tag="mv")
            st6 = constp.tile([128, G * 6], dt.float32, name="t_", tag="st6")
            std = constp.tile([128, G], dt.float32, name="t_", tag="stdv")
            rstd = constp.tile([128, G], dt.float32, name="t_", tag="rstd")
            nmt = constp.tile([128, G], dt.float32, name="t_", tag="nmt")

            agA = [dramp.tile([rowsA, H], dt.bfloat16, name=f"agA{l}", tag=f"agA{l}")
                   for l in range(n_layers - 1)]
            agB = [dramp.tile([rowsB, H], dt.bfloat16, name=f"agB{l}", tag=f"agB{l}")
                   for l in range(n_layers - 1)]
            tabA = [dramp.tile([NA, H], dt.bfloat16, name=f"tabA{l}", tag=f"tabA{l}")
                    for l in range(n_layers - 1)]
            tabB = [dramp.tile([NB, H], dt.bfloat16, name=f"tabB{l}", tag=f"tabB{l}")
                    for l in range(n_layers - 1)]

            qrr = [0]  # round-robin SWDGE queue counter
            pending_agb = [None]  # AG-B collective deferred into next layer

            for l in range(n_layers):
                with (
                    tc.tile_pool(name=f"gat{l}", bufs=10) as gatp,
                    tc.tile_pool(name=f"msg{l}", bufs=10) as msgp,
                    tc.tile_pool(name=f"sstr{l}", bufs=4) as sstrp,
                    tc.tile_pool(name=f"ntmp{l}", bufs=6) as ntp,
                    tc.tile_pool(name=f"eps{l}", bufs=2, space="PSUM") as epsp,
                    tc.tile_pool(name=f"agg{l}", bufs=2, space="PSUM") as aggpp,
                    tc.tile_pool(name=f"nps{l}", bufs=2, space="PSUM") as npsp,
                ):
                    msg_bufs = {}
                    next_chunk = {0: 0, 1: 0}

                    def emit_chunk(p, l=l):
                        k = next_chunk[p]
                        next_chunk[p] = k + 1
                        ts_, nt = chunks[p][k]
                        ne = nt * 128
                        gat = gatp.tile([128, CHT, H], dt.bfloat16,
                                        name="t_", tag="gat")
                        if l == 0:
                            nc.sync.dma_start(
                                gat[:, :nt, :].rearrange("p t f -> p (t f)"),
                                d_sg0[:, ts_ * H:(ts_ + nt) * H])
                        else:
                            src_ap = (tabB[l - 1][:, :] if p == 1
                                      else tabA[l - 1][:, :])
                            nc.gpsimd.dma_gather(
                                gat[:, :nt, :], src_ap,
                                gidx_sb[:, ts_ * 8:(ts_ + nt) * 8],
                                num_idxs=ne, num_idxs_reg=ne, elem_size=H,
                                queue_num=qrr[0] % NQ)
                            qrr[0] += 1
                        eps = epsp.tile([128, CHT * H], dt.float32,
                                        name="t_", tag="eps")
                        for i in range(nt):
                            t = ts_ + i
                            nc.tensor.matmul(
                                eps[:, bts(i, H)],
                                ea2_sb[:, bts(t // 2, 128)],
                                webe2_sb[:, bts(2 * l + (t % 2), H)],
                                start=True, stop=True)
                        msg = msgp.tile([128, CHT * H], dt.bfloat16,
                                        name="t_", tag="msg")
                        nc.vector.scalar_tensor_tensor(
                            msg[:, :nt * H], eps[:, :nt * H], 1.0 / WE_SCALE,
                            gat[:, :nt, :].rearrange("p t f -> p (t f)"),
                            OP.mult, OP.add)
                        nc.scalar.activation(msg[:, :nt * H], msg[:, :nt * H],
                                             AF.Relu)
                        msg_bufs[(p, k)] = msg

                    def msg_tile(t):
                        p, k, off = chunk_of[t]
                        while next_chunk[p] <= k:
                            emit_chunk(p)
                        return msg_bufs[(p, k)][:, bts(off, H)]

                    spilled = set()

                    def edge_quad(g, p):
                        if g not in by_group or p not in by_group[g]:
                            return
                        ts_, nt = by_group[g][p]
                        Ssb = sstrp.tile([128, max_nt * 128], dt.float8e4,
                                         name="t_", tag="sstr")
                        nc.sync.dma_start(Ssb[:, :nt * 128],
                                          d_S[:, ts_ * 128:(ts_ + nt) * 128])
                        aggt = aggpp.tile([128, H], dt.float32, name="t_", tag="agg")
                        for i, t in enumerate(range(ts_, ts_ + nt)):
                            m = msg_tile(t)
                            nc.tensor.matmul(
                                aggt[:], Ssb[:, bts(i, 128)], m,
                                start=(i == 0), stop=(i == nt - 1))
                        if g in spilled:
                            nc.vector.tensor_tensor(agg_sb[:, bts(g, H)],
                                                    agg_sb[:, bts(g, H)],
                                                    aggt[:], OP.add)
                        else:
                            nc.vector.tensor_copy(agg_sb[:, bts(g, H)], aggt[:])
                            spilled.add(g)

                    # -------- node phase helpers
                    def mlp_group(g):
                        s_g = s_node[:, bts(g, H)]
                        h = ntp.tile([128, H], dt.bfloat16, name="t_", tag="h")
                        nc.vector.tensor_tensor(h[:], s_g, agg_sb[:, bts(g, H)],
                                                OP.add)
                        htp = npsp.tile([128, H], dt.bfloat16, name="t_", tag="np")
                        nc.tensor.transpose(htp[:], h[:], ident_sb[:])
                        ht = ntp.tile([128, H], dt.bfloat16, name="t_", tag="ht")
                        nc.scalar.copy(ht[:], htp[:])
                        o1 = npsp.tile([128, H], dt.float32, name="t_", tag="np")
                        nc.tensor.matmul(o1[:], w1_sb[:, bts(l, H)], ht[:],
                                         start=True, stop=True)
                        x1 = ntp.tile([128, H], dt.bfloat16, name="t_", tag="x1")
                        nc.scalar.activation(x1[:], o1[:], AF.Silu,
                                             bias=b1_sb[:, l:l + 1])
                        o2 = npsp.tile([128, H], dt.float32, name="t_", tag="np")
                        nc.tensor.matmul(o2[:], x1[:], w2_sb[:, bts(l, H)],
                                         start=True, stop=False)
                        nc.tensor.matmul(o2[:], ones_sb[:1, :],
                                         b2r_sb[:1, bts(l, H)],
                                         start=False, stop=True)
                        nc.vector.tensor_tensor(s_g, s_g, o2[:], OP.add)
                        nc.vector.bn_stats(st6[:, g * 6:(g + 1) * 6], s_g)
                        nc.vector.bn_aggr(mv[:, g * 2:(g + 1) * 2],
                                          st6[:, g * 6:(g + 1) * 6])
                        nc.vector.tensor_scalar_add(veb[:, g:g + 1],
                                                    mv[:, g * 2 + 1:g * 2 + 2],
                                                    float(LN_EPS))

                    def ln_finish(g):
                        s_g = s_node[:, bts(g, H)]
                        nc.vector.scalar_tensor_tensor(
                            nmt[:, g:g + 1], mv[:, g * 2:g * 2 + 1], -1.0,
                            rstd[:, g:g + 1], OP.mult, OP.mult)
                        if trivial_ln and silu_native:
                            nc.scalar.activation(s_g, s_g, AF.Silu,
                                                 bias=nmt[:, g:g + 1],
                                                 scale=rstd[:, g:g + 1])
                        else:
                            xn = ntp.tile([128, H], dt.float32, name="t_", tag="xn")
                            nc.scalar.activation(xn[:], s_g, AF.Identity,
                                                 bias=nmt[:, g:g + 1],
                                                 scale=rstd[:, g:g + 1])
                            if not trivial_ln:
                                nc.vector.tensor_tensor(
                                    xn[:], xn[:], gb_sb[:, bts(l, H)], OP.mult)
                                nc.vector.tensor_tensor(
                                    xn[:], xn[:], bb_sb[:, bts(l, H)], OP.add)
                            if silu_native:
                                nc.scalar.activation(s_g, xn[:], AF.Silu)
                            else:
                                sg2 = ntp.tile([128, H], dt.float32, name="t_", tag="sg2")
                                nc.scalar.activation(sg2[:], xn[:], AF.Sigmoid)
                                nc.vector.tensor_tensor(s_g, xn[:], sg2[:], OP.mult)
                        if l < n_layers - 1:
                            tb16 = ntp.tile([128, H], dt.bfloat16, name="t_", tag="tb16")
                            nc.vector.tensor_tensor(tb16[:], s_g,
                                                    beb_sb[:, bts(l, H)], OP.add)
                            if g < ga:
                                nc.sync.dma_start(
                                    agA[l][g * 128:(g + 1) * 128, :], tb16[:])
                            else:
                                gg = g - ga
                                nc.sync.dma_start(
                                    agB[l][gg * 128:(gg + 1) * 128, :], tb16[:])
                        else:
                            nc.sync.dma_start(d_out[g * 128:(g + 1) * 128, :], s_g)

                    def node_sweep(gs, ge):
                        for g in range(gs, ge):
                            mlp_group(g)
                        ng = ge - gs
                        nc.scalar.activation(std[:, gs:gs + ng],
                                             veb[:, gs:gs + ng], AF.Sqrt)
                        nc.vector.reciprocal(rstd[:, gs:gs + ng],
                                             std[:, gs:gs + ng])
                        for g in range(gs, ge):
                            ln_finish(g)

                    # ---- quadrant-ordered layer body:
                    #  Q1 (phase A, low groups) | AG-B of prev layer |
                    #  Q2 (phase B, low) | node sweep low (overlaps Q3/Q4) |
                    #  Q3 (A, high) | Q4 (B, high) | AG-A | node sweep high
                    for g in range(0, ga):
                        edge_quad(g, 0)
                    if pending_agb[0] is not None:
                        pending_agb[0]()
                        pending_agb[0] = None
                    for g in range(0, ga):
                        edge_quad(g, 1)
                    node_sweep(0, ga)
                    for g in range(ga, G):
                        edge_quad(g, 0)
                    for g in range(ga, G):
                        edge_quad(g, 1)
                    for p in (0, 1):
                        while next_chunk[p] < len(chunks[p]):
                            emit_chunk(p)
                    if l < n_layers - 1:
                        nc.gpsimd.collective_compute(
                            "AllGather", mybir.AluOpType.bypass,
                            replica_groups=[list(range(n_cores))],
                            ins=[agA[l].opt()],
                            outs=[tabA[l].opt()])
                    node_sweep(ga, G)
                    msg_bufs.clear()

                if l < n_layers - 1:
                    def _agb(l=l):
                        nc.gpsimd.collective_compute(
                            "AllGather", mybir.AluOpType.bypass,
                            replica_groups=[list(range(n_cores))],
                            ins=[agB[l].opt()],
                            outs=[tabB[l].opt()])
                    pending_agb[0] = _agb

    nc.compile()
    return nc


# ================================================================== entrypoint
_CACHE = {}
TRACE = False
LAST_RESULT = None


def _setup_tracing():
    """Register the axon NTFF profile hook (dev/profiling only)."""
    import types
    if "antenv.axon_hooks" not in sys.modules:
        mod = types.ModuleType("antenv.axon_hooks")
        holder = [None]
        mod.get_axon_ntff_profile_hook = lambda: holder[0]
        mod.set_axon_ntff_profile_hook = lambda h: holder.__setitem__(0, h)
        sys.modules["antenv.axon_hooks"] = mod
        import antenv
        antenv.axon_hooks = mod
    try:
        from trn_agent_boot.trn_boot import _ntff_profile_via_ctypes
        hook = _ntff_profile_via_ctypes("/opt/axon/libaxon_pjrt.so")
        sys.modules["antenv.axon_hooks"].set_axon_ntff_profile_hook(hook)
    except Exception as e:  # degrade to no timing
        print("ntff hook setup failed:", e)
    import concourse.bass_utils as bu
    bu.upload_artifacts = lambda tmpdir: tmpdir


def _get_program(plan, trivial_ln):
    key = ("prog2", plan.NP, plan.Ttot, plan.ga,
           tuple((g, p, t) for (g, p, _, t) in plan.sched), trivial_ln)
    if key not in _CACHE:
        _CACHE[key] = build_program(plan, trivial_ln)
    return _CACHE[key]


def kernel(**inputs):
    s = np.asarray(inputs["s"], dtype=np.float32)
    edge_index = np.asarray(inputs["edge_index"])
    edge_attr = np.asarray(inputs["edge_attr"], dtype=np.float32)
    We = np.asarray(inputs["We"], dtype=np.float32)
    be = np.asarray(inputs["be"], dtype=np.float32)
    W1 = np.asarray(inputs["W1"], dtype=np.float32)
    b1 = np.asarray(inputs["b1"], dtype=np.float32)
    W2 = np.asarray(inputs["W2"], dtype=np.float32)
    b2 = np.asarray(inputs["b2"], dtype=np.float32)
    gamma = np.asarray(inputs["gamma"], dtype=np.float32)
    beta = np.asarray(inputs["beta"], dtype=np.float32)

    n = s.shape[0]
    plan = build_plan(edge_index, n)
    trivial_ln = bool(np.allclose(gamma, 1.0) and np.allclose(beta, 0.0))
    nc = _get_program(plan, trivial_ln)

    in_maps = [build_core_inputs(plan, c, s, edge_attr, We, be, W1, b1,
                                 W2, b2, gamma, beta, trivial_ln)
               for c in range(plan.n_cores)]

    if TRACE:
        _setup_tracing()
    from concourse.bass_utils import run_bass_kernel_spmd
    res = run_bass_kernel_spmd(nc, in_maps, core_ids=list(range(plan.n_cores)),
                               trace=TRACE)
    global LAST_RESULT
    LAST_RESULT = res
    out = np.concatenate([res.results[c]["out"] for c in range(plan.n_cores)],
                         axis=0)[:n]
    return np.ascontiguousarray(out.astype(np.float32))
